# revision 9
# baseline (speedup 1.0000x reference)
"""BinaryXnorExceptOutliersLinear on 8 Trainium2 NeuronCores — v2.

Reference math:
    mask, bscale from global kth-value quantiles of w
    w_q  = per-row asymmetric 8-bit fake quant of w
    w_sim = mask ? w_q : sign(w_q)*bscale
    out  = x @ w_sim.T + bias

The reference quant clips: codes saturate at zp_r + rng_r (~ the row
midpoint), so every w above c_r = fl(255*fl(rng/255) + zp) quantizes to the
per-row constant c_r; below-range clipping is impossible (zp << wmin).

v2 approximations (rel tolerance is 2e-2; both errors are far inside it):
  * outlier values use min(w, c_r) instead of w_q (|diff| <= qstep/2 on 5%
    of elements -> out err std ~0.2 vs abs budget ~6).
  * sign(w_q) = +1 iff w > whi_r (exact per-row f32 threshold from binary
    search, identical to reference rounding incl. the clip); the
    measure-zero w_q==0 band is folded into -1 (host prints a count; it is
    0 for this data).

Device per core (1024 out-rows):
    t     = (w>u)|(w<l) ? min(w, c_r) : -inf          (DVE pass 1)
    w_sim = t > -1e30 ? t : (w>whi_r ? bscale : -bscale)   (DVE pass 2)
            -- fp16 out
    DMA-xbar transpose w_sim into [i, o] panels of 512 out-columns
    out[b, o] accumulated by 64 chunk matmuls per panel:
        stationary xT chunk [i=128, b=32], moving panel [i=128, o=512]
    drain psum + bias (replicated [32, o] f32) on DVE, DMA out.

Sharding: weight rows (out_features) across 8 cores, x replicated; per-core
outputs [32, 1024] concatenated on host along features.
"""
import sys

sys.path.insert(0, "/opt/trn_rl_repo")

import numpy as np
from contextlib import ExitStack

import bass_rust
import concourse.bass as bass
import concourse.mybir as mybir
import concourse.tile as tile
from concourse.bass_utils import run_bass_kernel_spmd
from concourse import dve_ops
from concourse.dve_spec import (
    Spec, Src0, Src1, C0, C1, C2, C3, Zero, MaxNeg, lower, select, minn,
    _spill_c3_to_src1,
)
from concourse.dve_uop import DveOpSpec

# ---------------------------------------------------------------------------
OUT_F = 8192
IN_F = 8192
BATCH = 32
N_CORES = 8
ROWS_PER_CORE = OUT_F // N_CORES       # 1024
P = 128
BLKS = ROWS_PER_CORE // P              # 8 row-blocks per core
CH = IN_F // P                         # 64 i-chunks
NPC = 4                                # w processed in NPC pieces per block
PIECE = IN_F // NPC                    # 2048
PCH = PIECE // P                       # 16 i-chunks per piece
PANEL_BLKS = 4                         # row-blocks per psum panel
PANEL_W = PANEL_BLKS * P               # 512 out-columns per panel
N_PANELS = BLKS // PANEL_BLKS          # 2
OUTLIER_FRACTION = 0.05

f32 = mybir.dt.float32
f16 = mybir.dt.float16

# ---------------------------------------------------------------------------
# custom DVE op


def _register_op(name, spec):
    if name in dve_ops._SUB_OPCODE_FOR_NAME:
        return next(op for op in dve_ops.OPS if op.name == name)
    row = max(dve_ops._SUB_OPCODE_FOR_NAME.values()) + 1
    assert row < 0x20, "custom DVE row overflow"
    dve_ops._SUB_OPCODE_FOR_NAME[name] = row
    shas = {}
    for ver in ("v3", "v4"):
        uops = lower(spec, ver=ver)
        shas[ver] = DveOpSpec(
            name=name, opcode=row, uops=uops, rd1_en=dve_ops.has_src1(spec)
        ).sha(ver)
    op = dve_ops.DveOp(name=name, spec=spec, subdim=False, uops_sha=shas)
    dve_ops.OPS.append(op)
    dve_ops.CUSTOM_DVE_SPECS[name] = spec
    return op


# t = (w>u)|(w<l) ? min(w, c_r) : -inf
#   Src0=w f32; C0=u [P,1]; C1=l [P,1]; C3=c_r (in1 spill)
OP_TOUT = _register_op(
    "XNOR_TOUT",
    Spec(
        body=_spill_c3_to_src1(
            select((Src0 > C0) | (Src0 < C1), minn(Src0, C3), MaxNeg)
        ),
        reference=lambda in0, in1, s0, s1, imm2: np.where(
            (in0 > s0) | (in0 < s1), np.minimum(in0, in1),
            np.float32(np.finfo(np.float32).min),
        ).astype(np.float32),
    ),
)

# w_sim = t >= sent ? t : (w>whi ? bscale : -bscale)
#   Src0=t fp16; Src1=w f32; C0=whi_r [P,1]; C1=sent(-1e30) [P,1]; C2=bscale
OP_WSIM = _register_op(
    "XNOR_WSIM",
    Spec(
        body=select(Src0 >= C1, Src0,
                    select(Src1 > C0, C2, Zero - C2)),
        reference=lambda in0, in1, s0, s1, imm2: np.where(
            in0 >= s1, in0,
            np.where(in1 > s0, imm2, -imm2),
        ).astype(np.float32),
    ),
)

# ---------------------------------------------------------------------------
# walrus compatibility


def _prepare_for_walrus(nc):
    mybir.codegen_inst_isa_subclasses(nc)
    ctr = 0
    for bb in nc.main_func.blocks:
        new = []
        for inst in bb.instructions:
            si = inst.sync_info
            if si is not None and len(si.on_wait) > 1:
                waits = list(si.on_wait)
                for w in waits[:-1]:
                    nop = bass_rust.InstNoOp(
                        name=f"I-wsplit-{ctr}", engine=inst.engine
                    )
                    ctr += 1
                    nop.sync_info = mybir.SyncInfo(on_wait=[w], on_update=[])
                    try:
                        nc.register_instruction(nop, overwrite=True)
                    except Exception:
                        pass
                    new.append(nop)
                si.on_wait = [waits[-1]]
            new.append(inst)
        bb.instructions = new
    return nc


# ---------------------------------------------------------------------------
# device program


def _build_nc(bscale):
    nc = bass.Bass()
    wS = nc.dram_tensor("wS", [ROWS_PER_CORE, IN_F], f32, kind="ExternalInput")
    xT = nc.dram_tensor("xT", [IN_F, BATCH], f16, kind="ExternalInput")
    whiS = nc.dram_tensor("whiS", [P, BLKS], f32, kind="ExternalInput")
    cS = nc.dram_tensor("cS", [P, BLKS], f32, kind="ExternalInput")
    uT = nc.dram_tensor("uT", [P, 1], f32, kind="ExternalInput")
    lT = nc.dram_tensor("lT", [P, 1], f32, kind="ExternalInput")
    bR = nc.dram_tensor("bR", [BATCH, ROWS_PER_CORE], f32,
                        kind="ExternalInput")
    y = nc.dram_tensor("y", [BATCH, ROWS_PER_CORE], f32, kind="ExternalOutput")

    A = mybir.AluOpType

    with tile.TileContext(nc) as tc, ExitStack() as ctx:
        const_pool = ctx.enter_context(tc.tile_pool(name="const", bufs=1))
        wpool = ctx.enter_context(tc.tile_pool(name="w", bufs=2))
        mpool = ctx.enter_context(tc.tile_pool(name="m", bufs=2))
        wspool = ctx.enter_context(tc.tile_pool(name="ws", bufs=2))
        tpool = ctx.enter_context(tc.tile_pool(name="t", bufs=2))
        opool = ctx.enter_context(tc.tile_pool(name="o", bufs=2))
        psum = ctx.enter_context(tc.tile_pool(name="psum", bufs=2,
                                              space="PSUM"))

        # persistent loads
        xt16 = const_pool.tile([P, CH, BATCH], f16)
        nc.gpsimd.dma_start(xt16[:], xT.rearrange("(c p) b -> p c b", p=P))
        whi_t = const_pool.tile([P, BLKS], f32)
        nc.gpsimd.dma_start(whi_t[:], whiS[:])
        c_t = const_pool.tile([P, BLKS], f32)
        nc.gpsimd.dma_start(c_t[:], cS[:])
        u_t = const_pool.tile([P, 1], f32)
        nc.gpsimd.dma_start(u_t[:], uT[:])
        l_t = const_pool.tile([P, 1], f32)
        nc.gpsimd.dma_start(l_t[:], lT[:])
        sent_t = const_pool.tile([P, 1], f32)
        nc.vector.memset(sent_t[:], -1e30)
        bias_t = const_pool.tile([BATCH, ROWS_PER_CORE], f32)
        nc.gpsimd.dma_start(bias_t[:], bR[:])

        pan = None
        for blk in range(BLKS):
            if blk % PANEL_BLKS == 0:
                pan = tpool.tile([P, CH, PANEL_W], f16, tag="pan")
            po = (blk % PANEL_BLKS) * P
            for h in range(NPC):
                wt = wpool.tile([P, PIECE], f32, tag="wt")
                nc.gpsimd.dma_start(
                    wt[:],
                    wS[blk * P:(blk + 1) * P, h * PIECE:(h + 1) * PIECE],
                )
                mt = mpool.tile([P, PIECE], f16, tag="mt")
                nc.vector._custom_dve(
                    OP_TOUT, out=mt[:], in0=wt[:],
                    in1=c_t[:, blk:blk + 1],
                    s0=u_t[:], s1=l_t[:],
                )
                ws = wspool.tile([P, PIECE], f16, tag="ws")
                nc.vector._custom_dve(
                    OP_WSIM, out=ws[:], in0=mt[:], in1=wt[:],
                    s0=whi_t[:, blk:blk + 1], s1=sent_t[:],
                    imm2=float(bscale),
                )
                eng = nc.sync if (blk * NPC + h) % 2 == 0 else nc.scalar
                eng.dma_start_transpose(
                    pan[:, h * PCH:(h + 1) * PCH, po:po + P], ws[:]
                )
            if blk % PANEL_BLKS == PANEL_BLKS - 1:
                pi = blk // PANEL_BLKS
                ps = psum.tile([BATCH, PANEL_W], f32, tag="ps")
                for c in range(CH):
                    nc.tensor.matmul(
                        ps[:], xt16[:, c, :], pan[:, c, :],
                        start=(c == 0), stop=(c == CH - 1),
                    )
                osb = opool.tile([BATCH, PANEL_W], f32, tag="osb")
                nc.vector.scalar_tensor_tensor(
                    osb[:], ps[:], 1.0,
                    bias_t[:, pi * PANEL_W:(pi + 1) * PANEL_W],
                    A.mult, A.add,
                )
                nc.scalar.dma_start(
                    y[:, pi * PANEL_W:(pi + 1) * PANEL_W], osb[:]
                )

    _prepare_for_walrus(nc)
    return nc


# ---------------------------------------------------------------------------
# host precompute


def _sign_thresholds(wmin, wmax):
    """Per-row f32 threshold whi s.t. the reference's binarized sign
    sign_f32(q(w)*scale + zp) is +1 iff w > whi, for every f32 w, where
    q(w) = clip(rne(f32(f32(f32(w-zp)*255)/rng)),0,255).

    g(w) is monotone non-decreasing in w, so binary search over the f32
    bit lattice finds the exact boundary. Returns (whi, n_zero_rows) where
    n_zero_rows counts rows owning an exact w_q==0 code (folded into -1)."""
    rng = (wmax - wmin).astype(np.float32)
    zp = np.round(wmin - np.float32(128.0) * rng / np.float32(255.0)).astype(
        np.float32)
    scale = (rng / np.float32(255.0)).astype(np.float32)
    n = wmin.shape[0]

    def q_of_w(w):
        t = ((w - zp) * np.float32(255.0)).astype(np.float32)
        t = (t / rng).astype(np.float32)
        return np.clip(np.round(t), 0.0, 255.0).astype(np.float32)

    qs = np.arange(256, dtype=np.float32)
    gvals = (qs[None, :] * scale[:, None] + zp[:, None]).astype(np.float32)
    pos = gvals > 0
    neg = gvals < 0
    q_pos = np.where(pos.any(1), np.argmax(pos, 1), 256).astype(np.float32)
    q_neg = np.where(neg.any(1), 255 - np.argmax(neg[:, ::-1], 1), -1)
    n_zero = int(np.sum(q_pos - q_neg > 1.5))

    lo = np.full(n, np.float32(-1e30))
    hi = np.full(n, np.float32(1e30))

    def key(f):
        i = f.view(np.int32).astype(np.int64)
        return np.where(i < 0, -2147483648 - i, i)

    def unkey(k):
        i = np.where(k < 0, -2147483648 - k, k).astype(np.int64)
        return i.astype(np.int32).view(np.float32)

    klo, khi = key(lo), key(hi)
    for _ in range(64):
        kmid = (klo + khi) // 2
        wmid = unkey(kmid)
        below = q_of_w(wmid) < q_pos
        klo = np.where(below, kmid, klo)
        khi = np.where(below, khi, kmid)
        if (khi - klo <= 1).all():
            break
    return unkey(klo).astype(np.float32), n_zero


def _host_precompute(x, weight, bias):
    w = np.ascontiguousarray(weight, dtype=np.float32)
    n = w.size
    k_lo = int(n * OUTLIER_FRACTION / 2)
    k_hi = int(n * (1.0 - OUTLIER_FRACTION / 2))
    part = np.partition(w.reshape(-1), [k_lo - 1, k_hi - 1])
    lo = np.float32(part[k_lo - 1])
    hi = np.float32(part[k_hi - 1])
    keep = ~((w < lo) | (w > hi))
    bscale = np.float32(
        np.sum(np.abs(w) * keep, dtype=np.float32)
        / np.sum(keep, dtype=np.float32)
    )
    wmin = w.min(1).astype(np.float32)
    wmax = w.max(1).astype(np.float32)
    whi, n_zero = _sign_thresholds(wmin, wmax)
    if n_zero:
        print(f"kernel: {n_zero} rows own an exact w_q==0 code "
              "(folded into sign=-1)")
    # top-of-range clip value: w_q(w) == c for every w quantizing to code 255
    rng = (wmax - wmin).astype(np.float32)
    zp = np.round(wmin - np.float32(128.0) * rng / np.float32(255.0)).astype(
        np.float32)
    c = (np.float32(255.0) * (rng / np.float32(255.0)) + zp).astype(
        np.float32)

    x2 = np.ascontiguousarray(x, dtype=np.float32).reshape(BATCH, IN_F)
    xT16 = np.ascontiguousarray(x2.T).astype(np.float16)
    return w, xT16, whi, c, lo, hi, bscale


def _run(inputs, trace=False):
    x, weight, bias = inputs["x"], inputs["weight"], inputs["bias"]
    w, xT16, whi, cclip, lo, hi, bscale = _host_precompute(x, weight, bias)
    bias = np.ascontiguousarray(bias, np.float32)
    nc = _build_nc(bscale)
    u_arr = np.full((P, 1), hi, np.float32)
    l_arr = np.full((P, 1), lo, np.float32)
    in_maps = []
    for c in range(N_CORES):
        sl = slice(c * ROWS_PER_CORE, (c + 1) * ROWS_PER_CORE)
        whi_c = np.ascontiguousarray(
            whi[sl].reshape(BLKS, P).T)            # [P, BLKS]
        cclip_c = np.ascontiguousarray(
            cclip[sl].reshape(BLKS, P).T)          # [P, BLKS]
        bias_c = np.ascontiguousarray(
            np.broadcast_to(bias[sl], (BATCH, ROWS_PER_CORE)))
        in_maps.append({
            "wS": np.ascontiguousarray(w[sl]),
            "xT": xT16,
            "whiS": whi_c,
            "cS": cclip_c,
            "uT": u_arr,
            "lT": l_arr,
            "bR": bias_c,
        })
    res = run_bass_kernel_spmd(
        nc, in_maps, core_ids=list(range(N_CORES)), trace=trace
    )
    ys = np.concatenate([r["y"] for r in res.results], axis=1)  # [32, 8192]
    out = np.ascontiguousarray(ys).reshape(BATCH, 1, OUT_F).astype(np.float32)
    return out, res


def kernel(**inputs):
    out, _ = _run(inputs, trace=False)
    return out


# revision 20
# speedup vs baseline: 1.5865x; 1.5865x over previous
"""BinaryXnorExceptOutliersLinear on 8 Trainium2 NeuronCores — v2.

Reference math:
    mask, bscale from global kth-value quantiles of w
    w_q  = per-row asymmetric 8-bit fake quant of w
    w_sim = mask ? w_q : sign(w_q)*bscale
    out  = x @ w_sim.T + bias

The reference quant clips: codes saturate at zp_r + rng_r (~ the row
midpoint), so every w above c_r = fl(255*fl(rng/255) + zp) quantizes to the
per-row constant c_r; below-range clipping is impossible (zp << wmin).

v2 approximations (rel tolerance is 2e-2; both errors are far inside it):
  * outlier values use min(w, c_r) instead of w_q (|diff| <= qstep/2 on 5%
    of elements -> out err std ~0.2 vs abs budget ~6).
  * sign(w_q) = +1 iff w > whi_r (exact per-row f32 threshold from binary
    search, identical to reference rounding incl. the clip); the
    measure-zero w_q==0 band is folded into -1 (host prints a count; it is
    0 for this data).

Device per core (1024 out-rows), per row-block of 128, per half of in-dim:
    s     = Sign(w - whi_r)              (Scalar/ACT engine, fp16 out)
    s2    = s * bscale                   (DVE tensor_scalar, fp16)
    w_sim = (w>u)|(w<l) ? min(w, c_r) : s2   (one custom DVE op, fp16)
    DMA-xbar transpose w_sim -> mct [i=128, 32 chunks, o=128]
    out[b, o-block] += 32 chunk matmuls:
        stationary xT chunk [i=128, b=32], moving mct chunk [i=128, o=128]
        accumulating into a [32, 512] psum bank column slice
    drain psum + bias (replicated [32, o] f32) on DVE, DMA out.
Full-row-block DMA loads ([128, 8192] f32) keep 32 KB packets per
partition line -- small pieces measured only ~230 GB/s vs ~305 at 32 KB.

Sharding: weight rows (out_features) across 8 cores, x replicated; per-core
outputs [32, 1024] concatenated on host along features.
"""
import sys

sys.path.insert(0, "/opt/trn_rl_repo")

import numpy as np
from contextlib import ExitStack

import bass_rust
import concourse.bass as bass
import concourse.mybir as mybir
import concourse.tile as tile
from concourse.bass_utils import run_bass_kernel_spmd
from concourse import dve_ops
from concourse.dve_spec import (
    Spec, Src0, Src1, C0, C1, C2, lower, select, minn,
)
from concourse.dve_uop import DveOpSpec

# ---------------------------------------------------------------------------
OUT_F = 8192
IN_F = 8192
BATCH = 32
N_CORES = 8
ROWS_PER_CORE = OUT_F // N_CORES       # 1024
P = 128
BLKS = ROWS_PER_CORE // P              # 8 row-blocks per core
CH = IN_F // P                         # 64 i-chunks
HALF = IN_F // 2                       # 4096: fp16 stages work on half-blocks
HCH = HALF // P                        # 32 i-chunks per half
OUTLIER_FRACTION = 0.05

f32 = mybir.dt.float32
f16 = mybir.dt.float16

# ---------------------------------------------------------------------------
# custom DVE op


def _register_op(name, spec):
    if name in dve_ops._SUB_OPCODE_FOR_NAME:
        return next(op for op in dve_ops.OPS if op.name == name)
    row = max(dve_ops._SUB_OPCODE_FOR_NAME.values()) + 1
    assert row < 0x20, "custom DVE row overflow"
    dve_ops._SUB_OPCODE_FOR_NAME[name] = row
    shas = {}
    for ver in ("v3", "v4"):
        uops = lower(spec, ver=ver)
        shas[ver] = DveOpSpec(
            name=name, opcode=row, uops=uops, rd1_en=dve_ops.has_src1(spec)
        ).sha(ver)
    op = dve_ops.DveOp(name=name, spec=spec, subdim=False, uops_sha=shas)
    dve_ops.OPS.append(op)
    dve_ops.CUSTOM_DVE_SPECS[name] = spec
    return op


# w_sim = (w>u)|(w<l) ? min(w, c_r) : s2     (s2 = bscale*sign(w-whi))
#   Src0=w f32; Src1=s2 fp16; C0=c_r [P,1]; C1=u [P,1]; C2=l (imm)
OP_WSIM = _register_op(
    "XNOR_WSIM3",
    Spec(
        body=select((Src0 > C1) | (Src0 < C2), minn(Src0, C0), Src1),
        reference=lambda in0, in1, s0, s1, imm2: np.where(
            (in0 > s1) | (in0 < imm2), np.minimum(in0, s0), in1,
        ).astype(np.float32),
    ),
)

# ---------------------------------------------------------------------------
# walrus compatibility


def _prepare_for_walrus(nc):
    mybir.codegen_inst_isa_subclasses(nc)
    ctr = 0
    for bb in nc.main_func.blocks:
        new = []
        for inst in bb.instructions:
            si = inst.sync_info
            if si is not None and len(si.on_wait) > 1:
                waits = list(si.on_wait)
                for w in waits[:-1]:
                    nop = bass_rust.InstNoOp(
                        name=f"I-wsplit-{ctr}", engine=inst.engine
                    )
                    ctr += 1
                    nop.sync_info = mybir.SyncInfo(on_wait=[w], on_update=[])
                    try:
                        nc.register_instruction(nop, overwrite=True)
                    except Exception:
                        pass
                    new.append(nop)
                si.on_wait = [waits[-1]]
            new.append(inst)
        bb.instructions = new
    return nc


# ---------------------------------------------------------------------------
# device program


def _build_nc(bscale, l_imm):
    nc = bass.Bass()
    wS = nc.dram_tensor("wS", [ROWS_PER_CORE, IN_F], f32, kind="ExternalInput")
    xT = nc.dram_tensor("xT", [IN_F, BATCH], f16, kind="ExternalInput")
    nwhiS = nc.dram_tensor("nwhiS", [P, BLKS], f32, kind="ExternalInput")
    cS = nc.dram_tensor("cS", [P, BLKS], f32, kind="ExternalInput")
    uT = nc.dram_tensor("uT", [P, 1], f32, kind="ExternalInput")
    bR = nc.dram_tensor("bR", [BATCH, ROWS_PER_CORE], f32,
                        kind="ExternalInput")
    y = nc.dram_tensor("y", [BATCH, ROWS_PER_CORE], f32, kind="ExternalOutput")

    A = mybir.AluOpType
    Sign = mybir.ActivationFunctionType.Sign

    with tile.TileContext(nc) as tc, ExitStack() as ctx:
        const_pool = ctx.enter_context(tc.tile_pool(name="const", bufs=1))
        wpool = ctx.enter_context(tc.tile_pool(name="w", bufs=3))
        spool = ctx.enter_context(tc.tile_pool(name="s", bufs=2))
        s2pool = ctx.enter_context(tc.tile_pool(name="s2", bufs=2))
        wspool = ctx.enter_context(tc.tile_pool(name="ws", bufs=2))
        tpool = ctx.enter_context(tc.tile_pool(name="t", bufs=4))
        opool = ctx.enter_context(tc.tile_pool(name="o", bufs=2))
        psum = ctx.enter_context(tc.tile_pool(name="psum", bufs=2,
                                              space="PSUM"))

        # persistent loads
        xt16 = const_pool.tile([P, CH, BATCH], f16)
        nc.gpsimd.dma_start(xt16[:], xT.rearrange("(c p) b -> p c b", p=P))
        nwhi_t = const_pool.tile([P, BLKS], f32)
        nc.gpsimd.dma_start(nwhi_t[:], nwhiS[:])
        c_t = const_pool.tile([P, BLKS], f32)
        nc.gpsimd.dma_start(c_t[:], cS[:])
        u_t = const_pool.tile([P, 1], f32)
        nc.gpsimd.dma_start(u_t[:], uT[:])
        bias_t = const_pool.tile([BATCH, ROWS_PER_CORE], f32)
        nc.gpsimd.dma_start(bias_t[:], bR[:])

        ps_banks = [psum.tile([BATCH, 4 * P], f32, tag=f"ps{i}",
                              name=f"ps{i}")
                    for i in range(2)]
        for blk in range(BLKS):
            wt = wpool.tile([P, IN_F], f32, tag="wt")
            nc.gpsimd.dma_start(wt[:], wS[blk * P:(blk + 1) * P, :])
            ps = ps_banks[blk // 4]
            po = (blk % 4) * P
            for h in range(2):
                wh = wt[:, h * HALF:(h + 1) * HALF]
                st = spool.tile([P, HALF], f16, tag="st")
                nc.scalar.activation(
                    st[:], wh, Sign,
                    bias=nwhi_t[:, blk:blk + 1], scale=1.0,
                )
                s2 = s2pool.tile([P, HALF], f16, tag="s2")
                nc.vector.tensor_scalar(
                    s2[:], st[:], float(bscale), None, A.mult
                )
                ws = wspool.tile([P, HALF], f16, tag="ws")
                nc.vector._custom_dve(
                    OP_WSIM, out=ws[:], in0=wh, in1=s2[:],
                    s0=c_t[:, blk:blk + 1], s1=u_t[:], imm2=float(l_imm),
                )
                mct = tpool.tile([P, HCH, P], f16, tag="mct")
                nc.sync.dma_start_transpose(mct[:], ws[:])
                for cl in range(HCH):
                    c = h * HCH + cl
                    nc.tensor.matmul(
                        ps[:, po:po + P], xt16[:, c, :], mct[:, cl, :],
                        start=(c == 0), stop=(c == CH - 1),
                    )
            if blk % 4 == 3:
                pi = blk // 4
                osb = opool.tile([BATCH, 4 * P], f32, tag="osb")
                nc.vector.scalar_tensor_tensor(
                    osb[:], ps[:], 1.0,
                    bias_t[:, pi * 4 * P:(pi + 1) * 4 * P],
                    A.mult, A.add,
                )
                nc.gpsimd.dma_start(
                    y[:, pi * 4 * P:(pi + 1) * 4 * P], osb[:]
                )

    _prepare_for_walrus(nc)
    return nc


# ---------------------------------------------------------------------------
# host precompute


def _sign_thresholds(wmin, wmax):
    """Per-row f32 threshold whi s.t. the reference's binarized sign
    sign_f32(q(w)*scale + zp) is +1 iff w > whi, for every f32 w, where
    q(w) = clip(rne(f32(f32(f32(w-zp)*255)/rng)),0,255).

    g(w) is monotone non-decreasing in w, so binary search over the f32
    bit lattice finds the exact boundary. Returns (whi, n_zero_rows) where
    n_zero_rows counts rows owning an exact w_q==0 code (folded into -1)."""
    rng = (wmax - wmin).astype(np.float32)
    zp = np.round(wmin - np.float32(128.0) * rng / np.float32(255.0)).astype(
        np.float32)
    scale = (rng / np.float32(255.0)).astype(np.float32)
    n = wmin.shape[0]

    def q_of_w(w):
        t = ((w - zp) * np.float32(255.0)).astype(np.float32)
        t = (t / rng).astype(np.float32)
        return np.clip(np.round(t), 0.0, 255.0).astype(np.float32)

    qs = np.arange(256, dtype=np.float32)
    gvals = (qs[None, :] * scale[:, None] + zp[:, None]).astype(np.float32)
    pos = gvals > 0
    neg = gvals < 0
    q_pos = np.where(pos.any(1), np.argmax(pos, 1), 256).astype(np.float32)
    q_neg = np.where(neg.any(1), 255 - np.argmax(neg[:, ::-1], 1), -1)
    n_zero = int(np.sum(q_pos - q_neg > 1.5))

    lo = np.full(n, np.float32(-1e30))
    hi = np.full(n, np.float32(1e30))

    def key(f):
        i = f.view(np.int32).astype(np.int64)
        return np.where(i < 0, -2147483648 - i, i)

    def unkey(k):
        i = np.where(k < 0, -2147483648 - k, k).astype(np.int64)
        return i.astype(np.int32).view(np.float32)

    klo, khi = key(lo), key(hi)
    for _ in range(64):
        kmid = (klo + khi) // 2
        wmid = unkey(kmid)
        below = q_of_w(wmid) < q_pos
        klo = np.where(below, kmid, klo)
        khi = np.where(below, khi, kmid)
        if (khi - klo <= 1).all():
            break
    return unkey(klo).astype(np.float32), n_zero


def _host_precompute(x, weight, bias):
    w = np.ascontiguousarray(weight, dtype=np.float32)
    n = w.size
    k_lo = int(n * OUTLIER_FRACTION / 2)
    k_hi = int(n * (1.0 - OUTLIER_FRACTION / 2))
    part = np.partition(w.reshape(-1), [k_lo - 1, k_hi - 1])
    lo = np.float32(part[k_lo - 1])
    hi = np.float32(part[k_hi - 1])
    keep = ~((w < lo) | (w > hi))
    bscale = np.float32(
        np.sum(np.abs(w) * keep, dtype=np.float32)
        / np.sum(keep, dtype=np.float32)
    )
    wmin = w.min(1).astype(np.float32)
    wmax = w.max(1).astype(np.float32)
    whi, n_zero = _sign_thresholds(wmin, wmax)
    if n_zero:
        print(f"kernel: {n_zero} rows own an exact w_q==0 code "
              "(folded into sign=-1)")
    # top-of-range clip value: w_q(w) == c for every w quantizing to code 255
    rng = (wmax - wmin).astype(np.float32)
    zp = np.round(wmin - np.float32(128.0) * rng / np.float32(255.0)).astype(
        np.float32)
    c = (np.float32(255.0) * (rng / np.float32(255.0)) + zp).astype(
        np.float32)

    x2 = np.ascontiguousarray(x, dtype=np.float32).reshape(BATCH, IN_F)
    xT16 = np.ascontiguousarray(x2.T).astype(np.float16)
    return w, xT16, whi, c, lo, hi, bscale


def _run(inputs, trace=False):
    x, weight, bias = inputs["x"], inputs["weight"], inputs["bias"]
    w, xT16, whi, cclip, lo, hi, bscale = _host_precompute(x, weight, bias)
    bias = np.ascontiguousarray(bias, np.float32)
    nc = _build_nc(bscale, lo)
    u_arr = np.full((P, 1), hi, np.float32)
    nwhi = (-whi).astype(np.float32)
    in_maps = []
    for c in range(N_CORES):
        sl = slice(c * ROWS_PER_CORE, (c + 1) * ROWS_PER_CORE)
        nwhi_c = np.ascontiguousarray(
            nwhi[sl].reshape(BLKS, P).T)           # [P, BLKS]
        cclip_c = np.ascontiguousarray(
            cclip[sl].reshape(BLKS, P).T)          # [P, BLKS]
        bias_c = np.ascontiguousarray(
            np.broadcast_to(bias[sl], (BATCH, ROWS_PER_CORE)))
        in_maps.append({
            "wS": np.ascontiguousarray(w[sl]),
            "xT": xT16,
            "nwhiS": nwhi_c,
            "cS": cclip_c,
            "uT": u_arr,
            "bR": bias_c,
        })
    res = run_bass_kernel_spmd(
        nc, in_maps, core_ids=list(range(N_CORES)), trace=trace
    )
    ys = np.concatenate([r["y"] for r in res.results], axis=1)  # [32, 8192]
    out = np.ascontiguousarray(ys).reshape(BATCH, 1, OUT_F).astype(np.float32)
    return out, res


def kernel(**inputs):
    out, _ = _run(inputs, trace=False)
    return out


# revision 24
# speedup vs baseline: 1.7416x; 1.0978x over previous
"""BinaryXnorExceptOutliersLinear on 8 Trainium2 NeuronCores — v2.

Reference math:
    mask, bscale from global kth-value quantiles of w
    w_q  = per-row asymmetric 8-bit fake quant of w
    w_sim = mask ? w_q : sign(w_q)*bscale
    out  = x @ w_sim.T + bias

The reference quant clips: codes saturate at zp_r + rng_r (~ the row
midpoint), so every w above c_r = fl(255*fl(rng/255) + zp) quantizes to the
per-row constant c_r; below-range clipping is impossible (zp << wmin).

v2 approximations (rel tolerance is 2e-2; both errors are far inside it):
  * outlier values use min(w, c_r) instead of w_q (|diff| <= qstep/2 on 5%
    of elements -> out err std ~0.2 vs abs budget ~6).
  * sign(w_q) = +1 iff w > whi_r (exact per-row f32 threshold from binary
    search, identical to reference rounding incl. the clip); the
    measure-zero w_q==0 band is folded into -1 (host prints a count; it is
    0 for this data).

Device per core (1024 out-rows), per row-block of 128, per half of in-dim:
    s     = Sign(w - whi_r)              (Scalar/ACT engine, fp16 out)
    s2    = s * bscale                   (DVE tensor_scalar, fp16)
    w_sim = (w>u)|(w<l) ? min(w, c_r) : s2   (one custom DVE op, fp16)
    DMA-xbar transpose w_sim -> mct [i=128, 32 chunks, o=128]
    out[b, o-block] += 32 chunk matmuls:
        stationary xT chunk [i=128, b=32], moving mct chunk [i=128, o=128]
        accumulating into a [32, 512] psum bank column slice
    drain psum + bias (replicated [32, o] f32) on DVE, DMA out.
Full-row-block DMA loads ([128, 8192] f32) keep 32 KB packets per
partition line -- small pieces measured only ~230 GB/s vs ~305 at 32 KB.

Sharding: weight rows (out_features) across 8 cores, x replicated; per-core
outputs [32, 1024] concatenated on host along features.
"""
import sys

sys.path.insert(0, "/opt/trn_rl_repo")

import numpy as np
from contextlib import ExitStack

import bass_rust
import concourse.bass as bass
import concourse.mybir as mybir
import concourse.tile as tile
from concourse.bass_utils import run_bass_kernel_spmd
from concourse import dve_ops
from concourse.dve_spec import (
    Spec, Src0, Src1, C0, C1, C2, lower, select, minn,
)
from concourse.dve_uop import DveOpSpec

# ---------------------------------------------------------------------------
OUT_F = 8192
IN_F = 8192
BATCH = 32
N_CORES = 8
ROWS_PER_CORE = OUT_F // N_CORES       # 1024
P = 128
BLKS = ROWS_PER_CORE // P              # 8 row-blocks per core
CH = IN_F // P                         # 64 i-chunks
HALF = IN_F // 2                       # 4096: fp16 stages work on half-blocks
HCH = HALF // P                        # 32 i-chunks per half
OUTLIER_FRACTION = 0.05

f32 = mybir.dt.float32
f16 = mybir.dt.float16

# ---------------------------------------------------------------------------
# custom DVE op


def _register_op(name, spec):
    if name in dve_ops._SUB_OPCODE_FOR_NAME:
        return next(op for op in dve_ops.OPS if op.name == name)
    row = max(dve_ops._SUB_OPCODE_FOR_NAME.values()) + 1
    assert row < 0x20, "custom DVE row overflow"
    dve_ops._SUB_OPCODE_FOR_NAME[name] = row
    shas = {}
    for ver in ("v3", "v4"):
        uops = lower(spec, ver=ver)
        shas[ver] = DveOpSpec(
            name=name, opcode=row, uops=uops, rd1_en=dve_ops.has_src1(spec)
        ).sha(ver)
    op = dve_ops.DveOp(name=name, spec=spec, subdim=False, uops_sha=shas)
    dve_ops.OPS.append(op)
    dve_ops.CUSTOM_DVE_SPECS[name] = spec
    return op


# w_sim = (w>u)|(w<l) ? min(w, c_r) : s2     (s2 = bscale*sign(w-whi))
#   Src0=w f32; Src1=s2 fp16; C0=c_r [P,1]; C1=u [P,1]; C2=l (imm)
OP_WSIM = _register_op(
    "XNOR_WSIM3",
    Spec(
        body=select((Src0 > C1) | (Src0 < C2), minn(Src0, C0), Src1),
        reference=lambda in0, in1, s0, s1, imm2: np.where(
            (in0 > s1) | (in0 < imm2), np.minimum(in0, s0), in1,
        ).astype(np.float32),
    ),
)

# ---------------------------------------------------------------------------
# walrus compatibility


def _prepare_for_walrus(nc):
    mybir.codegen_inst_isa_subclasses(nc)
    ctr = 0
    for bb in nc.main_func.blocks:
        new = []
        for inst in bb.instructions:
            si = inst.sync_info
            if si is not None and len(si.on_wait) > 1:
                waits = list(si.on_wait)
                for w in waits[:-1]:
                    nop = bass_rust.InstNoOp(
                        name=f"I-wsplit-{ctr}", engine=inst.engine
                    )
                    ctr += 1
                    nop.sync_info = mybir.SyncInfo(on_wait=[w], on_update=[])
                    try:
                        nc.register_instruction(nop, overwrite=True)
                    except Exception:
                        pass
                    new.append(nop)
                si.on_wait = [waits[-1]]
            new.append(inst)
        bb.instructions = new
    return nc


# ---------------------------------------------------------------------------
# device program


def _build_nc(bscale, l_imm):
    nc = bass.Bass()
    wS = nc.dram_tensor("wS", [ROWS_PER_CORE, IN_F], f32, kind="ExternalInput")
    xT = nc.dram_tensor("xT", [IN_F, BATCH], f16, kind="ExternalInput")
    nwhiS = nc.dram_tensor("nwhiS", [P, BLKS], f32, kind="ExternalInput")
    cS = nc.dram_tensor("cS", [P, BLKS], f32, kind="ExternalInput")
    uT = nc.dram_tensor("uT", [P, 1], f32, kind="ExternalInput")
    bP = nc.dram_tensor("bP", [P, BLKS], f32, kind="ExternalInput")
    y = nc.dram_tensor("y", [ROWS_PER_CORE, BATCH], f32, kind="ExternalOutput")

    A = mybir.AluOpType
    Sign = mybir.ActivationFunctionType.Sign

    with tile.TileContext(nc) as tc, ExitStack() as ctx:
        const_pool = ctx.enter_context(tc.tile_pool(name="const", bufs=1))
        wpool = ctx.enter_context(tc.tile_pool(name="w", bufs=3))
        spool = ctx.enter_context(tc.tile_pool(name="s", bufs=2))
        s2pool = ctx.enter_context(tc.tile_pool(name="s2", bufs=2))
        wspool = ctx.enter_context(tc.tile_pool(name="ws", bufs=2))
        tpool = ctx.enter_context(tc.tile_pool(name="t", bufs=4))
        opool = ctx.enter_context(tc.tile_pool(name="o", bufs=2))
        psum = ctx.enter_context(tc.tile_pool(name="psum", bufs=2,
                                              space="PSUM"))

        # persistent loads
        xt16 = const_pool.tile([P, CH, BATCH], f16)
        nc.gpsimd.dma_start(xt16[:], xT.rearrange("(c p) b -> p c b", p=P))
        nwhi_t = const_pool.tile([P, BLKS], f32)
        nc.gpsimd.dma_start(nwhi_t[:], nwhiS[:])
        c_t = const_pool.tile([P, BLKS], f32)
        nc.gpsimd.dma_start(c_t[:], cS[:])
        u_t = const_pool.tile([P, 1], f32)
        nc.gpsimd.dma_start(u_t[:], uT[:])
        bias_t = const_pool.tile([P, BLKS], f32)
        nc.gpsimd.dma_start(bias_t[:], bP[:])

        for blk in range(BLKS):
            wt = wpool.tile([P, IN_F], f32, tag="wt")
            nc.gpsimd.dma_start(wt[:], wS[blk * P:(blk + 1) * P, :])
            ps = psum.tile([P, BATCH], f32, tag="ps")
            for h in range(2):
                wh = wt[:, h * HALF:(h + 1) * HALF]
                st = spool.tile([P, HALF], f16, tag="st")
                nc.scalar.activation(
                    st[:], wh, Sign,
                    bias=nwhi_t[:, blk:blk + 1], scale=1.0,
                )
                s2 = s2pool.tile([P, HALF], f16, tag="s2")
                nc.vector.tensor_scalar(
                    s2[:], st[:], float(bscale), None, A.mult
                )
                ws = wspool.tile([P, HALF], f16, tag="ws")
                nc.vector._custom_dve(
                    OP_WSIM, out=ws[:], in0=wh, in1=s2[:],
                    s0=c_t[:, blk:blk + 1], s1=u_t[:], imm2=float(l_imm),
                )
                mct = tpool.tile([P, HCH, P], f16, tag="mct")
                nc.sync.dma_start_transpose(mct[:], ws[:])
                for cl in range(HCH):
                    c = h * HCH + cl
                    nc.tensor.matmul(
                        ps[:], mct[:, cl, :], xt16[:, c, :],
                        start=(c == 0), stop=(c == CH - 1),
                    )
            osb = opool.tile([P, BATCH], f32, tag="osb")
            nc.scalar.activation(
                osb[:], ps[:], mybir.ActivationFunctionType.Identity,
                bias=bias_t[:, blk:blk + 1], scale=1.0,
            )
            nc.gpsimd.dma_start(y[blk * P:(blk + 1) * P, :], osb[:])

    _prepare_for_walrus(nc)
    return nc


# ---------------------------------------------------------------------------
# host precompute


def _sign_thresholds(wmin, wmax):
    """Per-row f32 threshold whi s.t. the reference's binarized sign
    sign_f32(q(w)*scale + zp) is +1 iff w > whi, for every f32 w, where
    q(w) = clip(rne(f32(f32(f32(w-zp)*255)/rng)),0,255).

    g(w) is monotone non-decreasing in w, so binary search over the f32
    bit lattice finds the exact boundary. Returns (whi, n_zero_rows) where
    n_zero_rows counts rows owning an exact w_q==0 code (folded into -1)."""
    rng = (wmax - wmin).astype(np.float32)
    zp = np.round(wmin - np.float32(128.0) * rng / np.float32(255.0)).astype(
        np.float32)
    scale = (rng / np.float32(255.0)).astype(np.float32)
    n = wmin.shape[0]

    def q_of_w(w):
        t = ((w - zp) * np.float32(255.0)).astype(np.float32)
        t = (t / rng).astype(np.float32)
        return np.clip(np.round(t), 0.0, 255.0).astype(np.float32)

    qs = np.arange(256, dtype=np.float32)
    gvals = (qs[None, :] * scale[:, None] + zp[:, None]).astype(np.float32)
    pos = gvals > 0
    neg = gvals < 0
    q_pos = np.where(pos.any(1), np.argmax(pos, 1), 256).astype(np.float32)
    q_neg = np.where(neg.any(1), 255 - np.argmax(neg[:, ::-1], 1), -1)
    n_zero = int(np.sum(q_pos - q_neg > 1.5))

    lo = np.full(n, np.float32(-1e30))
    hi = np.full(n, np.float32(1e30))

    def key(f):
        i = f.view(np.int32).astype(np.int64)
        return np.where(i < 0, -2147483648 - i, i)

    def unkey(k):
        i = np.where(k < 0, -2147483648 - k, k).astype(np.int64)
        return i.astype(np.int32).view(np.float32)

    klo, khi = key(lo), key(hi)
    for _ in range(64):
        kmid = (klo + khi) // 2
        wmid = unkey(kmid)
        below = q_of_w(wmid) < q_pos
        klo = np.where(below, kmid, klo)
        khi = np.where(below, khi, kmid)
        if (khi - klo <= 1).all():
            break
    return unkey(klo).astype(np.float32), n_zero


def _host_precompute(x, weight, bias):
    w = np.ascontiguousarray(weight, dtype=np.float32)
    n = w.size
    k_lo = int(n * OUTLIER_FRACTION / 2)
    k_hi = int(n * (1.0 - OUTLIER_FRACTION / 2))
    part = np.partition(w.reshape(-1), [k_lo - 1, k_hi - 1])
    lo = np.float32(part[k_lo - 1])
    hi = np.float32(part[k_hi - 1])
    keep = ~((w < lo) | (w > hi))
    bscale = np.float32(
        np.sum(np.abs(w) * keep, dtype=np.float32)
        / np.sum(keep, dtype=np.float32)
    )
    wmin = w.min(1).astype(np.float32)
    wmax = w.max(1).astype(np.float32)
    whi, n_zero = _sign_thresholds(wmin, wmax)
    if n_zero:
        print(f"kernel: {n_zero} rows own an exact w_q==0 code "
              "(folded into sign=-1)")
    # top-of-range clip value: w_q(w) == c for every w quantizing to code 255
    rng = (wmax - wmin).astype(np.float32)
    zp = np.round(wmin - np.float32(128.0) * rng / np.float32(255.0)).astype(
        np.float32)
    c = (np.float32(255.0) * (rng / np.float32(255.0)) + zp).astype(
        np.float32)

    x2 = np.ascontiguousarray(x, dtype=np.float32).reshape(BATCH, IN_F)
    xT16 = np.ascontiguousarray(x2.T).astype(np.float16)
    return w, xT16, whi, c, lo, hi, bscale


def _run(inputs, trace=False):
    x, weight, bias = inputs["x"], inputs["weight"], inputs["bias"]
    w, xT16, whi, cclip, lo, hi, bscale = _host_precompute(x, weight, bias)
    bias = np.ascontiguousarray(bias, np.float32)
    nc = _build_nc(bscale, lo)
    u_arr = np.full((P, 1), hi, np.float32)
    nwhi = (-whi).astype(np.float32)
    in_maps = []
    for c in range(N_CORES):
        sl = slice(c * ROWS_PER_CORE, (c + 1) * ROWS_PER_CORE)
        nwhi_c = np.ascontiguousarray(
            nwhi[sl].reshape(BLKS, P).T)           # [P, BLKS]
        cclip_c = np.ascontiguousarray(
            cclip[sl].reshape(BLKS, P).T)          # [P, BLKS]
        bias_c = np.ascontiguousarray(
            bias[sl].reshape(BLKS, P).T)           # [P, BLKS]
        in_maps.append({
            "wS": np.ascontiguousarray(w[sl]),
            "xT": xT16,
            "nwhiS": nwhi_c,
            "cS": cclip_c,
            "uT": u_arr,
            "bP": bias_c,
        })
    res = run_bass_kernel_spmd(
        nc, in_maps, core_ids=list(range(N_CORES)), trace=trace
    )
    ys = np.concatenate([r["y"] for r in res.results], axis=0)  # [8192, 32]
    out = np.ascontiguousarray(ys.T).reshape(BATCH, 1, OUT_F).astype(
        np.float32)
    return out, res


def kernel(**inputs):
    out, _ = _run(inputs, trace=False)
    return out
